# revision 8
# baseline (speedup 1.0000x reference)
"""Trainium2 Bass kernel for nn_Direction: out = input @ qr(weight + 1e-8).Q.T

Strategy (data-parallel over 8 NeuronCores):
  - Host: Q = np.linalg.qr(weight + 1e-8).Q  (512x26, tiny; LAPACK Householder
    matches the jnp.linalg.qr sign convention). Replicated to all cores.
  - Host: shard input [262144, 26] by batch into 8 x [32768, 26], and hand each
    core its shard pre-transposed as [26, 32768] so the contraction dim (26) is
    the SBUF partition dim - the layout the PE matmul needs for lhsT.
  - Device (per core): for each 128-row batch tile j,
        psum[128, 512] = lhsT(xt[:, j*128:(j+1)*128]).T @ rhs(qt[26, 512])
    with float32r (fp32 bits, full-rate PE mode at N=512), copy PSUM->SBUF on
    DVE/ACT alternately (DMA cannot read PSUM on TRN2), and DMA 2 MiB staged
    chunks of the output back to HBM.
  - Host: concatenate the 8 x [32768, 512] shards.
"""

import sys

import numpy as np

try:
    import concourse  # noqa: F401
except ImportError:
    sys.path.insert(0, "/opt/trn_rl_repo")

from concourse import bacc, mybir, tile
from concourse.bass_utils import run_bass_kernel_spmd

N_CORES = 8
B = 262144
D = 26
OUT = 512
ROWS = B // N_CORES  # 32768 batch rows per core

MM = 128  # batch rows per matmul (PSUM partition dim)
STAGE = 4  # matmul tiles per staged output DMA (4 * 256 KiB = 1 MiB)
GROUP = STAGE * MM  # 1024 batch rows per staged output DMA
# Input DMA chunk sizes (batch rows). Graduated: small first chunks so the
# first matmuls (and therefore the output DMA stream) start ~5us earlier
# than with a uniform 8192 split; 26-partition input DMAs are port-limited.
CHUNKS = [1024, 3072, 4096, 8192, 8192, 8192]
assert sum(CHUNKS) == ROWS and all(c % GROUP == 0 for c in CHUNKS)

_F32 = mybir.dt.float32
_F32R = mybir.dt.float32r

_NC = None


def _emit(tc, xt, qt, out):
    nc = tc.nc
    with (
        tc.tile_pool(name="qt", bufs=1) as qt_pool,
        tc.tile_pool(name="xt", bufs=3) as xt_pool,
        tc.tile_pool(name="stage", bufs=4) as stage_pool,
        tc.tile_pool(name="psum", bufs=8, space="PSUM") as psum_pool,
    ):
        qt_sb = qt_pool.tile([D, OUT], _F32R)
        nc.scalar.dma_start(qt_sb[:], qt[:])
        row = 0
        for chunk in CHUNKS:
            xt_sb = xt_pool.tile([D, max(CHUNKS)], _F32R, tag="xt_sb")
            nc.scalar.dma_start(xt_sb[:, :chunk], xt[:, row : row + chunk])
            for g in range(chunk // GROUP):
                stage = stage_pool.tile([MM, STAGE * OUT], _F32)
                for t in range(STAGE):
                    j = g * STAGE + t
                    ps = psum_pool.tile([MM, OUT], _F32)
                    nc.tensor.matmul(
                        ps[:],
                        xt_sb[:, j * MM : (j + 1) * MM],
                        qt_sb[:],
                    )
                    dst = stage[:, t * OUT : (t + 1) * OUT]
                    if t % 2 == 0:
                        nc.vector.tensor_copy(dst, ps[:])
                    else:
                        nc.scalar.copy(dst, ps[:])
                base = row + g * GROUP
                out_view = out[base : base + GROUP, :].rearrange(
                    "(t p) o -> p t o", p=MM
                )
                stage_view = stage[:].rearrange("p (t o) -> p t o", t=STAGE)
                nc.sync.dma_start(out_view, stage_view)
            row += chunk


def _build():
    global _NC
    if _NC is not None:
        return _NC
    nc = bacc.Bacc(
        "TRN2", target_bir_lowering=False, debug=False, num_devices=N_CORES
    )
    xt = nc.dram_tensor("xt", [D, ROWS], _F32R, kind="ExternalInput").ap()
    qt = nc.dram_tensor("qt", [D, OUT], _F32R, kind="ExternalInput").ap()
    out = nc.dram_tensor("out", [ROWS, OUT], _F32, kind="ExternalOutput").ap()
    with tile.TileContext(nc) as tc:
        _emit(tc, xt, qt, out)
    nc.compile()
    _NC = nc
    return nc


def _run(in_maps, trace=False, **kwargs):
    nc = _build()
    return run_bass_kernel_spmd(
        nc, in_maps, list(range(N_CORES)), trace=trace, **kwargs
    )


def _prepare_in_maps(input, weight):
    x = np.asarray(input, dtype=np.float32)
    w = np.asarray(weight, dtype=np.float32)
    assert x.shape == (B, D) and w.shape == (OUT, D)
    q, _ = np.linalg.qr(w + np.float32(1e-8))
    qt = np.ascontiguousarray(q.T, dtype=np.float32)  # [26, 512]
    return [
        {
            "xt": np.ascontiguousarray(x[c * ROWS : (c + 1) * ROWS].T),
            "qt": qt,
        }
        for c in range(N_CORES)
    ]


def kernel(input, weight):
    in_maps = _prepare_in_maps(input, weight)
    res = _run(in_maps)
    return np.concatenate([r["out"] for r in res.results], axis=0)


# revision 11
# speedup vs baseline: 1.0088x; 1.0088x over previous
"""Trainium2 Bass kernel for nn_Direction: out = input @ qr(weight + 1e-8).Q.T

Strategy (data-parallel over 8 NeuronCores):
  - Host: Q = np.linalg.qr(weight + 1e-8).Q  (512x26, tiny; LAPACK Householder
    matches the jnp.linalg.qr sign convention). Replicated to all cores.
  - Host: shard input [262144, 26] by batch into 8 x [32768, 26], and hand each
    core its shard pre-transposed as [26, 32768] so the contraction dim (26) is
    the SBUF partition dim - the layout the PE matmul needs for lhsT.
  - Device (per core): for each 128-row batch tile j,
        psum[128, 512] = lhsT(xt[:, j*128:(j+1)*128]).T @ rhs(qt[26, 512])
    with float32r (fp32 bits, full-rate PE mode at N=512), copy PSUM->SBUF on
    DVE/ACT alternately (DMA cannot read PSUM on TRN2), and DMA 2 MiB staged
    chunks of the output back to HBM.
  - Host: concatenate the 8 x [32768, 512] shards.
"""

import sys

import numpy as np

try:
    import concourse  # noqa: F401
except ImportError:
    sys.path.insert(0, "/opt/trn_rl_repo")

from concourse import bacc, mybir, tile
from concourse.bass_utils import run_bass_kernel_spmd

N_CORES = 8
B = 262144
D = 26
OUT = 512
ROWS = B // N_CORES  # 32768 batch rows per core

MM = 128  # batch rows per matmul (PSUM partition dim)
STAGE = 8  # matmul tiles per staged output DMA (8 * 256 KiB = 2 MiB)
GROUP = STAGE * MM  # 1024 batch rows per staged output DMA
# Input DMA chunk sizes (batch rows). Graduated: small first chunks so the
# first matmuls (and therefore the output DMA stream) start ~5us earlier
# than with a uniform 8192 split; 26-partition input DMAs are port-limited.
CHUNKS = [1024, 3072, 4096, 8192, 8192, 8192]
assert sum(CHUNKS) == ROWS and all(c % GROUP == 0 for c in CHUNKS)

_F32 = mybir.dt.float32
_F32R = mybir.dt.float32r

_NC = None


def _emit(tc, xt, qt, out):
    nc = tc.nc
    with (
        tc.tile_pool(name="qt", bufs=1) as qt_pool,
        tc.tile_pool(name="xt", bufs=3) as xt_pool,
        tc.tile_pool(name="stage", bufs=4) as stage_pool,
        tc.tile_pool(name="psum", bufs=8, space="PSUM") as psum_pool,
    ):
        qt_sb = qt_pool.tile([D, OUT], _F32R)
        nc.scalar.dma_start(qt_sb[:], qt[:])
        row = 0
        for ci, chunk in enumerate(CHUNKS):
            xt_sb = xt_pool.tile([D, max(CHUNKS)], _F32R, tag="xt_sb")
            nc.scalar.dma_start(xt_sb[:, :chunk], xt[:, row : row + chunk])
            # Small first staging groups so the output DMA stream starts as
            # soon as the first couple of (cold-PE) matmuls finish.
            if ci == 0:
                stages = [2, 2, 4] + [STAGE] * ((chunk // MM - 8) // STAGE)
            else:
                stages = [STAGE] * (chunk // MM // STAGE)
            assert sum(stages) * MM == chunk
            j = 0
            for n_tiles in stages:
                stage = stage_pool.tile([MM, STAGE * OUT], _F32, tag="stage")
                for t in range(n_tiles):
                    ps = psum_pool.tile([MM, OUT], _F32)
                    nc.tensor.matmul(
                        ps[:],
                        xt_sb[:, (j + t) * MM : (j + t + 1) * MM],
                        qt_sb[:],
                    )
                    dst = stage[:, t * OUT : (t + 1) * OUT]
                    if t % 2 == 0:
                        nc.vector.tensor_copy(dst, ps[:])
                    else:
                        nc.scalar.copy(dst, ps[:])
                base = row + j * MM
                out_view = out[base : base + n_tiles * MM, :].rearrange(
                    "(t p) o -> p t o", p=MM
                )
                stage_view = stage[:, : n_tiles * OUT].rearrange(
                    "p (t o) -> p t o", t=n_tiles
                )
                nc.sync.dma_start(out_view, stage_view)
                j += n_tiles
            row += chunk


def _build():
    global _NC
    if _NC is not None:
        return _NC
    nc = bacc.Bacc(
        "TRN2", target_bir_lowering=False, debug=False, num_devices=N_CORES
    )
    xt = nc.dram_tensor("xt", [D, ROWS], _F32R, kind="ExternalInput").ap()
    qt = nc.dram_tensor("qt", [D, OUT], _F32R, kind="ExternalInput").ap()
    out = nc.dram_tensor("out", [ROWS, OUT], _F32, kind="ExternalOutput").ap()
    with tile.TileContext(nc) as tc:
        _emit(tc, xt, qt, out)
    nc.compile()
    _NC = nc
    return nc


def _run(in_maps, trace=False, **kwargs):
    nc = _build()
    return run_bass_kernel_spmd(
        nc, in_maps, list(range(N_CORES)), trace=trace, **kwargs
    )


def _prepare_in_maps(input, weight):
    x = np.asarray(input, dtype=np.float32)
    w = np.asarray(weight, dtype=np.float32)
    assert x.shape == (B, D) and w.shape == (OUT, D)
    q, _ = np.linalg.qr(w + np.float32(1e-8))
    qt = np.ascontiguousarray(q.T, dtype=np.float32)  # [26, 512]
    return [
        {
            "xt": np.ascontiguousarray(x[c * ROWS : (c + 1) * ROWS].T),
            "qt": qt,
        }
        for c in range(N_CORES)
    ]


def kernel(input, weight):
    in_maps = _prepare_in_maps(input, weight)
    try:
        res = _run(in_maps)
    except Exception:
        # One retry: the axon-proxied execute path can transiently report
        # NRT_EXEC_UNIT_UNRECOVERABLE; the next run succeeds.
        res = _run(in_maps)
    return np.concatenate([r["out"] for r in res.results], axis=0)


# revision 18
# speedup vs baseline: 1.0139x; 1.0050x over previous
"""Trainium2 Bass kernel for nn_Direction: out = input @ qr(weight + 1e-8).Q.T

Strategy (data-parallel over 8 NeuronCores):
  - Host: Q = np.linalg.qr(weight + 1e-8).Q  (512x26, tiny; LAPACK Householder
    matches the jnp.linalg.qr sign convention). Replicated to all cores.
  - Host: shard input [262144, 26] by batch into 8 x [32768, 26], and hand each
    core its shard pre-transposed as [26, 32768] so the contraction dim (26) is
    the SBUF partition dim - the layout the PE matmul needs for lhsT.
  - Device (per core): for each 128-row batch tile j,
        psum[128, 512] = lhsT(xt[:, j*128:(j+1)*128]).T @ rhs(qt[26, 512])
    with float32r (fp32 bits, full-rate PE mode at N=512), copy PSUM->SBUF on
    DVE/ACT alternately (DMA cannot read PSUM on TRN2), and DMA 2 MiB staged
    chunks of the output back to HBM.
  - Host: concatenate the 8 x [32768, 512] shards.
"""

import sys

import numpy as np

try:
    import concourse  # noqa: F401
except ImportError:
    sys.path.insert(0, "/opt/trn_rl_repo")

from concourse import bacc, mybir, tile
from concourse.bass_utils import run_bass_kernel_spmd

N_CORES = 8
B = 262144
D = 26
OUT = 512
ROWS = B // N_CORES  # 32768 batch rows per core

MM = 128  # batch rows per matmul (PSUM partition dim)
STAGE = 8  # matmul tiles per staged output DMA (8 * 256 KiB = 2 MiB)
# Input DMA chunk sizes (batch rows) and per-chunk staging-group sizes (in
# 128-row matmul tiles). Graduated: a tiny first chunk and single-tile first
# staging groups get the first output bytes onto the (saturating) output DMA
# stream as early as possible; 26-partition input DMAs are port-limited, so
# big chunks amortize them once the stream is rolling.
CHUNKS = [256, 768, 3072, 4096, 8192, 8192, 8192]
STAGES = [[1, 1], [2, 4], [8] * 3, [8] * 4, [8] * 8, [8] * 8, [8] * 8]
assert sum(CHUNKS) == ROWS
assert all(sum(s) * MM == c for s, c in zip(STAGES, CHUNKS))

_F32 = mybir.dt.float32
_F32R = mybir.dt.float32r

_NC = None


def _emit(tc, xt, qt, out):
    nc = tc.nc
    with (
        tc.tile_pool(name="qt", bufs=1) as qt_pool,
        tc.tile_pool(name="xt", bufs=3) as xt_pool,
        tc.tile_pool(name="stage", bufs=5) as stage_pool,
        tc.tile_pool(name="psum", bufs=8, space="PSUM") as psum_pool,
    ):
        qt_sb = qt_pool.tile([D, OUT], _F32R)
        nc.gpsimd.dma_start(qt_sb[:], qt[:])
        row = 0
        for chunk, stages in zip(CHUNKS, STAGES):
            xt_sb = xt_pool.tile([D, max(CHUNKS)], _F32R, tag="xt_sb")
            # All input loads go via GpSimd SWDGE: the late chunks' dma_starts
            # wait on xt slot reuse, and on a compute engine that wait would
            # head-of-line-block the PSUM->SBUF copies queued behind it in the
            # engine's FIFO, starving the output stream (measured 1.6-2us
            # stalls). GpSimd has no other work. It also empirically starts
            # its first descriptor earlier than the ScalarE HWDGE path.
            nc.gpsimd.dma_start(xt_sb[:, :chunk], xt[:, row : row + chunk])
            j = 0
            for n_tiles in stages:
                stage = stage_pool.tile([MM, STAGE * OUT], _F32, tag="stage")
                for t in range(n_tiles):
                    ps = psum_pool.tile([MM, OUT], _F32)
                    nc.tensor.matmul(
                        ps[:],
                        xt_sb[:, (j + t) * MM : (j + t + 1) * MM],
                        qt_sb[:],
                    )
                    dst = stage[:, t * OUT : (t + 1) * OUT]
                    if t % 2 == 0:
                        nc.vector.tensor_copy(dst, ps[:])
                    else:
                        nc.scalar.copy(dst, ps[:])
                base = row + j * MM
                out_view = out[base : base + n_tiles * MM, :].rearrange(
                    "(t p) o -> p t o", p=MM
                )
                stage_view = stage[:, : n_tiles * OUT].rearrange(
                    "p (t o) -> p t o", t=n_tiles
                )
                nc.sync.dma_start(out_view, stage_view)
                j += n_tiles
            row += chunk


def _build():
    global _NC
    if _NC is not None:
        return _NC
    nc = bacc.Bacc(
        "TRN2",
        target_bir_lowering=False,
        debug=False,
        num_devices=N_CORES,
        enable_partition_id=False,
    )
    xt = nc.dram_tensor("xt", [D, ROWS], _F32R, kind="ExternalInput").ap()
    qt = nc.dram_tensor("qt", [D, OUT], _F32R, kind="ExternalInput").ap()
    out = nc.dram_tensor("out", [ROWS, OUT], _F32, kind="ExternalOutput").ap()
    with tile.TileContext(nc) as tc:
        _emit(tc, xt, qt, out)
    nc.compile()
    _NC = nc
    return nc


def _run(in_maps, trace=False, **kwargs):
    nc = _build()
    return run_bass_kernel_spmd(
        nc, in_maps, list(range(N_CORES)), trace=trace, **kwargs
    )


def _prepare_in_maps(input, weight):
    x = np.asarray(input, dtype=np.float32)
    w = np.asarray(weight, dtype=np.float32)
    assert x.shape == (B, D) and w.shape == (OUT, D)
    q, _ = np.linalg.qr(w + np.float32(1e-8))
    qt = np.ascontiguousarray(q.T, dtype=np.float32)  # [26, 512]
    return [
        {
            "xt": np.ascontiguousarray(x[c * ROWS : (c + 1) * ROWS].T),
            "qt": qt,
        }
        for c in range(N_CORES)
    ]


def kernel(input, weight):
    in_maps = _prepare_in_maps(input, weight)
    try:
        res = _run(in_maps)
    except Exception:
        # One retry: the axon-proxied execute path can transiently report
        # NRT_EXEC_UNIT_UNRECOVERABLE; the next run succeeds.
        res = _run(in_maps)
    return np.concatenate([r["out"] for r in res.results], axis=0)


# revision 23
# speedup vs baseline: 1.0267x; 1.0127x over previous
"""Trainium2 Bass kernel for nn_Direction: out = input @ qr(weight + 1e-8).Q.T

Strategy (data-parallel over 8 NeuronCores):
  - Host: Q = np.linalg.qr(weight + 1e-8).Q  (512x26, tiny; LAPACK Householder
    matches the jnp.linalg.qr sign convention). Replicated to all cores.
  - Host: shard input [262144, 26] by batch into 8 x [32768, 26]; each shard is
    pre-transposed so the contraction dim (26) is the SBUF partition dim, and
    packed as two 26-row partition groups [52, 16384] so the device can load
    it on two disjoint SBUF port groups concurrently (see GROUPS/POFF below).
  - Device (per core): for each 128-row batch tile j,
        psum[128, 512] = lhsT(xt_slice[26, 128]).T @ rhs(qt[26, 512])
    with float32r (fp32 bits, full-rate PE mode at N=512), copy PSUM->SBUF on
    DVE/ACT alternately (DMA cannot read PSUM on TRN2), and DMA 2 MiB staged
    chunks of the output back to HBM on the SyncE HWDGE ring, which stays
    saturated at the per-core HBM write bandwidth for the whole run.
  - Host: concatenate the 8 x [32768, 512] shards.
"""

import sys

import numpy as np

try:
    import concourse  # noqa: F401
except ImportError:
    sys.path.insert(0, "/opt/trn_rl_repo")

from concourse import bacc, mybir, tile
from concourse.bass_utils import run_bass_kernel_spmd

N_CORES = 8
B = 262144
D = 26
OUT = 512
ROWS = B // N_CORES  # 32768 batch rows per core

MM = 128  # batch rows per matmul (PSUM partition dim)
STAGE = 8  # matmul tiles per staged output DMA (8 * 256 KiB = 2 MiB)
# The per-core input shard is host-packed as [52, 16384]: two 26-row groups
# stacked on the partition axis (group g rows 26g..26g+25 hold batch rows
# g*16384..(g+1)*16384 transposed). On device the two groups live at SBUF
# partition offsets 0 and 64 (matmul operands must sit at base partition
# 0/32/64), so each input chunk is two concurrent [26, N] DMAs on disjoint
# port groups - 2x the port bandwidth of a single [26, N] destination. The
# whole 3.3 MiB shard stays SBUF-resident, so every load completes during
# the pipeline ramp instead of stealing HBM bandwidth from the output
# stream mid-run. Matmuls read lhsT/rhs at partition offset 64*g against a
# 2x-replicated qt.
GROUPS = 2
POFF = 64  # partition offset of group 1 in SBUF
GCOLS = ROWS // GROUPS  # 16384 batch rows (columns) per partition group
# Input DMA chunks, in columns of the packed layout. Graduated: a small
# first chunk so the first matmul starts early.
CHUNKS = [512, 1536, 3072, 3072, 4096, 4096]
assert sum(CHUNKS) == GCOLS
# Staging-group sizes (in 128-row matmul tiles) for the flat 256-tile loop:
# single-tile first groups so the output DMA stream starts as early as
# possible, 2 MiB groups in steady state.
STAGES = [1, 1, 2, 4] + [STAGE] * 31
assert sum(STAGES) * MM == ROWS

_F32 = mybir.dt.float32
_F32R = mybir.dt.float32r

_NC = None


def _emit(tc, xt, qt, out):
    nc = tc.nc
    with (
        tc.tile_pool(name="qt", bufs=1) as qt_pool,
        tc.tile_pool(name="xt", bufs=1) as xt_pool,
        tc.tile_pool(name="stage", bufs=5) as stage_pool,
        tc.tile_pool(name="psum", bufs=8, space="PSUM") as psum_pool,
    ):
        qt_sb = qt_pool.tile([POFF + D, OUT], _F32R)
        # Input loads go via GpSimd SWDGE so they never sit in a compute
        # engine's FIFO in front of PSUM->SBUF copies (measured 1.6-2us
        # output-stream stalls when they did). One SBUF-resident tile per
        # chunk (bufs=1, no slot reuse) keeps every dma_start wait-free.
        # SWDGE descriptor generation is serialized on the Q7 (~0.65us per
        # dma_start), so the two tensors the very first matmul needs
        # (chunk0/group0, qt/group0) generate first. After that, group-0 and
        # group-1 loads stay pairwise interleaved: the pair members land on
        # disjoint SBUF port groups (partitions 0-25 vs 64-89) and transfer
        # concurrently, which is what doubles the input port bandwidth.
        chunk_tiles = []
        col = 0
        for ci, chunk in enumerate(CHUNKS):
            ct = xt_pool.tile([POFF + D, chunk], _F32R, tag=f"xt{ci}")
            nc.gpsimd.dma_start(ct[:D, :], xt[:D, col : col + chunk])
            if ci == 0:
                nc.gpsimd.dma_start(qt_sb[:D, :], qt[:D, :])
            nc.gpsimd.dma_start(
                ct[POFF : POFF + D, :], xt[D:, col : col + chunk]
            )
            if ci == 0:
                nc.gpsimd.dma_start(qt_sb[POFF : POFF + D, :], qt[D:, :])
            chunk_tiles.append((col, col + chunk, ct))
            col += chunk
        j = 0
        for n_tiles in STAGES:
            stage = stage_pool.tile([MM, STAGE * OUT], _F32, tag="stage")
            for t in range(n_tiles):
                g, jj = divmod(j + t, GCOLS // MM)
                c0 = jj * MM
                base_col, _, ct = next(
                    (a, b, x) for a, b, x in chunk_tiles if a <= c0 < b
                )
                po = g * POFF
                ps = psum_pool.tile([MM, OUT], _F32)
                nc.tensor.matmul(
                    ps[:],
                    ct[po : po + D, c0 - base_col : c0 - base_col + MM],
                    qt_sb[po : po + D, :],
                )
                dst = stage[:, t * OUT : (t + 1) * OUT]
                if t % 2 == 0:
                    nc.vector.tensor_copy(dst, ps[:])
                else:
                    nc.scalar.copy(dst, ps[:])
            base = j * MM
            out_view = out[base : base + n_tiles * MM, :].rearrange(
                "(t p) o -> p t o", p=MM
            )
            stage_view = stage[:, : n_tiles * OUT].rearrange(
                "p (t o) -> p t o", t=n_tiles
            )
            nc.sync.dma_start(out_view, stage_view)
            j += n_tiles


def _build():
    global _NC
    if _NC is not None:
        return _NC
    nc = bacc.Bacc(
        "TRN2",
        target_bir_lowering=False,
        debug=False,
        num_devices=N_CORES,
        enable_partition_id=False,
    )
    xt = nc.dram_tensor("xt", [GROUPS * D, GCOLS], _F32R, kind="ExternalInput").ap()
    qt = nc.dram_tensor("qt", [GROUPS * D, OUT], _F32R, kind="ExternalInput").ap()
    out = nc.dram_tensor("out", [ROWS, OUT], _F32, kind="ExternalOutput").ap()
    with tile.TileContext(nc) as tc:
        _emit(tc, xt, qt, out)
    nc.compile()
    _NC = nc
    return nc


def _run(in_maps, trace=False, **kwargs):
    nc = _build()
    return run_bass_kernel_spmd(
        nc, in_maps, list(range(N_CORES)), trace=trace, **kwargs
    )


def _prepare_in_maps(input, weight):
    x = np.asarray(input, dtype=np.float32)
    w = np.asarray(weight, dtype=np.float32)
    assert x.shape == (B, D) and w.shape == (OUT, D)
    q, _ = np.linalg.qr(w + np.float32(1e-8))
    qt = np.ascontiguousarray(np.tile(q.T, (GROUPS, 1)), dtype=np.float32)
    maps = []
    for c in range(N_CORES):
        shard = x[c * ROWS : (c + 1) * ROWS]  # [32768, 26]
        xt = np.empty((GROUPS * D, GCOLS), dtype=np.float32)
        for g in range(GROUPS):
            xt[g * D : (g + 1) * D] = shard[g * GCOLS : (g + 1) * GCOLS].T
        maps.append({"xt": xt, "qt": qt})
    return maps


def kernel(input, weight):
    in_maps = _prepare_in_maps(input, weight)
    try:
        res = _run(in_maps)
    except Exception:
        # One retry: the axon-proxied execute path can transiently report
        # NRT_EXEC_UNIT_UNRECOVERABLE; the next run succeeds.
        res = _run(in_maps)
    return np.concatenate([r["out"] for r in res.results], axis=0)
